# revision 7
# baseline (speedup 1.0000x reference)
"""Trainium2 Bass kernel for nn_DenseBayesian (dense + hard LWTA grouped argmax mask).

Computes out = x @ W.T + b, then per group of U=4 output units keeps only the
argmax unit (others zeroed). Data-parallel over 8 NeuronCores along the row axis.

Numerics: the matmul runs as an fp16x3 split product (x = xh + xl, W.T = wh + wl
in fp16; out = xl@wh + xh@wl + xh@wh accumulated in fp32 PSUM). fp16 x fp16
products are exact in fp32, so the result carries ~22 effective mantissa bits of
the inputs - the same accuracy class as native fp32 MACs - while streaming the
PE at 1 cycle/row (fp32 matmuls cost 4).

Self-contained: hardcodes the problem shapes; only needs numpy + the concourse
runtime available on the host.
"""
import numpy as np

import concourse.bass as bass
import concourse.mybir as mybir
import concourse.tile as tile
from concourse import bacc
from concourse.bass_utils import run_bass_kernel_spmd

f32 = mybir.dt.float32
f16 = mybir.dt.float16

N = 262144
DIN = 256
DOUT = 512
U = 4
NCORES = 8
ROWS = N // NCORES          # 32768 rows per core
MACRO = 256                 # rows per macro-tile (2 psum banks of 128 rows)
P = 128
KC = DIN // P               # k chunks
G = DOUT // U               # groups per row (128)


def build_program(n_macros: int, with_bias: bool, mul_engine: str = "pool"):
    """One NeuronCore program: n_macros macro-tiles of 256 rows each.

    mul_engine: "pool" puts the final mask-multiply on GpSimd (frees DVE),
    "dve" keeps everything on the vector engine (fallback).
    """
    nc = bacc.Bacc("TRN2", target_bir_lowering=False)
    rows = n_macros * MACRO

    xh_d = nc.dram_tensor("xh", [n_macros, P, KC, MACRO], f16, kind="ExternalInput")
    xl_d = nc.dram_tensor("xl", [n_macros, P, KC, MACRO], f16, kind="ExternalInput")
    wh_d = nc.dram_tensor("wh", [P, KC, DOUT], f16, kind="ExternalInput")
    wl_d = nc.dram_tensor("wl", [P, KC, DOUT], f16, kind="ExternalInput")
    if with_bias:
        bh_d = nc.dram_tensor("bh", [1, DOUT], f16, kind="ExternalInput")
        bl_d = nc.dram_tensor("bl", [1, DOUT], f16, kind="ExternalInput")
    out_d = nc.dram_tensor("out", [rows, DOUT], f32, kind="ExternalOutput")

    with tile.TileContext(nc) as tc:
        with tc.tile_pool(name="wpool", bufs=1) as wpool, \
             tc.tile_pool(name="xpool", bufs=4) as xpool, \
             tc.tile_pool(name="upool", bufs=3) as upool, \
             tc.tile_pool(name="mpool", bufs=3) as mpool, \
             tc.tile_pool(name="cpool", bufs=3) as cpool, \
             tc.tile_pool(name="opool", bufs=3) as opool, \
             tc.tile_pool(name="pspool", bufs=3, space="PSUM") as pspool:

            wh = wpool.tile([P, KC, DOUT], f16)
            nc.sync.dma_start(wh[:], wh_d[:])
            wl = wpool.tile([P, KC, DOUT], f16)
            nc.sync.dma_start(wl[:], wl_d[:])
            if with_bias:
                bh = wpool.tile([1, DOUT], f16)
                nc.sync.dma_start(bh[:], bh_d[:])
                bl = wpool.tile([1, DOUT], f16)
                nc.sync.dma_start(bl[:], bl_d[:])
                ones = wpool.tile([1, P], f16)
                nc.gpsimd.memset(ones[:], 1.0)

            for mt in range(n_macros):
                xh_t = xpool.tile([P, KC, MACRO], f16, tag="xh")
                nc.sync.dma_start(xh_t[:], xh_d[mt, :, :, :])
                xl_t = xpool.tile([P, KC, MACRO], f16, tag="xl")
                nc.sync.dma_start(xl_t[:], xl_d[mt, :, :, :])

                ps = pspool.tile([P, 2 * DOUT], f32)
                for s in range(2):
                    acc = ps[:, s * DOUT:(s + 1) * DOUT]
                    mms = []
                    if with_bias:
                        mms.append((ones[:, :], bh[:, :]))
                        mms.append((ones[:, :], bl[:, :]))
                    rs = slice(s * P, (s + 1) * P)
                    for (xa, wb) in ((xl_t, wh), (xh_t, wl), (xh_t, wh)):
                        for c in range(KC):
                            mms.append((xa[:, c, rs], wb[:, c, :]))
                    last = len(mms) - 1
                    for i, (lhsT, rhs) in enumerate(mms):
                        nc.tensor.matmul(acc, lhsT, rhs,
                                         start=(i == 0), stop=(i == last))

                # u = logits for 256 rows in SLOT-BLOCKED column order:
                # [p, (h, s, g)] where h=row-half, s=slot(0..3), g=group(0..127).
                # (W columns were permuted host-side: dout 4g+s -> block s*128+g,
                # so every masking op below reads contiguous 128-runs.)
                u = upool.tile([P, 2 * DOUT], f32)
                nc.scalar.activation(u[:], ps[:], mybir.ActivationFunctionType.Copy)

                u4 = u[:].rearrange("p (h s g) -> p h s g", h=2, s=U)
                # grouped max: 3 pairwise maxes over contiguous slot blocks
                mA = mpool.tile([P, 2, G], f32, tag="mA")
                nc.vector.tensor_tensor(mA[:], u4[:, :, 0, :], u4[:, :, 1, :],
                                        mybir.AluOpType.max)
                mB = mpool.tile([P, 2, G], f32, tag="mB")
                nc.vector.tensor_tensor(mB[:], u4[:, :, 2, :], u4[:, :, 3, :],
                                        mybir.AluOpType.max)
                m = mpool.tile([P, 2, G], f32, tag="m")
                nc.vector.tensor_tensor(m[:], mA[:], mB[:], mybir.AluOpType.max)

                # winner mask: u >= group-max (slot dim broadcast, inner contiguous)
                mb = m[:].unsqueeze(2).broadcast_to([P, 2, U, G])
                cmp = cpool.tile([P, 2, U, G], f32)
                nc.vector.tensor_tensor(cmp[:], u4, mb, mybir.AluOpType.is_ge)

                o = opool.tile([P, 2 * DOUT], f32)
                mul_eng = nc.gpsimd if mul_engine == "pool" else nc.vector
                mul_eng.tensor_tensor(o[:], u[:], cmp[:].rearrange("p h s g -> p (h s g)"),
                                      mybir.AluOpType.mult)

                # un-permute to standard dout order on ScalarE:
                # o is [p,(h,s,g)]; o_std is [p,(h,g,s)] = [p, h, j=4g+s]
                o_std = opool.tile([P, 2 * DOUT], f32, tag="o_std")
                nc.scalar.activation(
                    o_std[:].rearrange("p (h g s) -> p h g s", h=2, s=U),
                    o[:].rearrange("p (h s g) -> p h g s", h=2, s=U),
                    mybir.ActivationFunctionType.Copy)

                dst = out_d[mt * MACRO:(mt + 1) * MACRO, :].rearrange(
                    "(s p) j -> p s j", p=P)
                nc.sync.dma_start(dst, o_std[:].rearrange("p (s j) -> p s j", s=2))

    nc.compile()
    return nc


_programs: dict = {}


def _get_program(n_macros: int, with_bias: bool, mul_engine: str = "pool"):
    key = (n_macros, with_bias, mul_engine)
    if key not in _programs:
        _programs[key] = build_program(n_macros, with_bias, mul_engine)
    return _programs[key]


def _split_fp16(a: np.ndarray):
    hi = a.astype(np.float16)
    lo = (a - hi.astype(np.float32)).astype(np.float16)
    return hi, lo


def _pack_b(b: np.ndarray):
    """[DOUT] fp32 -> (hi, lo) [1, DOUT] fp16, slot-block permuted like W."""
    bp = b.astype(np.float32).reshape(G, U).T.reshape(1, DOUT)
    return _split_fp16(np.ascontiguousarray(bp))


def _pack_x(xs: np.ndarray, n_macros: int):
    """[rows, DIN] fp32 -> (hi, lo) tiled [n_macros, P, KC, MACRO] fp16."""
    hi, lo = _split_fp16(xs)
    packs = []
    for a in (hi, lo):
        # [rows, DIN] -> transpose -> k = c*P + p ; row = mt*MACRO + r
        at = np.ascontiguousarray(a.T)                      # [DIN, rows]
        at = at.reshape(KC, P, n_macros, MACRO)             # [c, p, mt, r]
        packs.append(np.ascontiguousarray(at.transpose(2, 1, 0, 3)))
    return packs


def _pack_w(W: np.ndarray):
    """[DOUT, DIN] fp32 -> (hi, lo) tiled [P, KC, DOUT] fp16 of W.T with
    slot-blocked column permutation: dout j=4g+s -> position s*G+g."""
    wT = W.astype(np.float32).T                             # [DIN, DOUT]
    # permute columns: new[:, s*G+g] = old[:, 4g+s]
    wTp = wT.reshape(DIN, G, U).transpose(0, 2, 1).reshape(DIN, DOUT)
    hi, lo = _split_fp16(np.ascontiguousarray(wTp))
    packs = []
    for a in (hi, lo):
        packs.append(np.ascontiguousarray(a.reshape(KC, P, DOUT).transpose(1, 0, 2)))
    return packs


def kernel(x: np.ndarray, W: np.ndarray, b: np.ndarray) -> np.ndarray:
    x = np.asarray(x, dtype=np.float32)
    W = np.asarray(W, dtype=np.float32)
    b = np.asarray(b, dtype=np.float32)
    assert x.shape == (N, DIN) and W.shape == (DOUT, DIN) and b.shape == (DOUT,)

    with_bias = bool(np.any(b))
    n_macros = ROWS // MACRO
    nc = _get_program(n_macros, with_bias)

    return _run(nc, x, W, b, with_bias, n_macros)


def _run(nc, x, W, b, with_bias, n_macros):

    wh, wl = _pack_w(W)
    in_maps = []
    for i in range(NCORES):
        xs = x[i * ROWS:(i + 1) * ROWS]
        xh, xl = _pack_x(xs, n_macros)
        im = {"xh": xh, "xl": xl, "wh": wh, "wl": wl}
        if with_bias:
            bhi, blo = _pack_b(b)
            im["bh"] = bhi
            im["bl"] = blo
        in_maps.append(im)

    res = run_bass_kernel_spmd(nc, in_maps, list(range(NCORES)))
    return np.concatenate([res.results[i]["out"] for i in range(NCORES)], axis=0)


# revision 8
# speedup vs baseline: 1.2930x; 1.2930x over previous
"""Trainium2 Bass kernel for nn_DenseBayesian (dense + hard LWTA grouped argmax mask).

Computes out = x @ W.T + b, then per group of U=4 output units keeps only the
argmax unit (others zeroed). Data-parallel over 8 NeuronCores along the row axis.

Numerics: the matmul runs as an fp16x3 split product (x = xh + xl, W.T = wh + wl
in fp16; out = xl@wh + xh@wl + xh@wh accumulated in fp32 PSUM). fp16 x fp16
products are exact in fp32, so the result carries ~22 effective mantissa bits of
the inputs - the same accuracy class as native fp32 MACs - while streaming the
PE at 1 cycle/row (fp32 matmuls cost 4).

Self-contained: hardcodes the problem shapes; only needs numpy + the concourse
runtime available on the host.
"""
import numpy as np

import concourse.bass as bass
import concourse.mybir as mybir
import concourse.tile as tile
from concourse import bacc
from concourse.bass_utils import run_bass_kernel_spmd

f32 = mybir.dt.float32
f16 = mybir.dt.float16

N = 262144
DIN = 256
DOUT = 512
U = 4
NCORES = 8
ROWS = N // NCORES          # 32768 rows per core
MACRO = 256                 # rows per macro-tile (2 psum banks of 128 rows)
P = 128
KC = DIN // P               # k chunks
G = DOUT // U               # groups per row (128)


def build_program(n_macros: int, with_bias: bool, mask_mode: str = "pool_sub"):
    """One NeuronCore program: n_macros macro-tiles of 256 rows each.

    mask_mode: "pool_sub" = subtract on GpSimd + fused (d>=0)*u on DVE;
    "dve_cmp" = is_ge on DVE + multiply on GpSimd.
    """
    nc = bacc.Bacc("TRN2", target_bir_lowering=False)
    rows = n_macros * MACRO

    xh_d = nc.dram_tensor("xh", [n_macros, P, KC, MACRO], f16, kind="ExternalInput")
    xl_d = nc.dram_tensor("xl", [n_macros, P, KC, MACRO], f16, kind="ExternalInput")
    wh_d = nc.dram_tensor("wh", [P, KC, DOUT], f16, kind="ExternalInput")
    wl_d = nc.dram_tensor("wl", [P, KC, DOUT], f16, kind="ExternalInput")
    if with_bias:
        bh_d = nc.dram_tensor("bh", [1, DOUT], f16, kind="ExternalInput")
        bl_d = nc.dram_tensor("bl", [1, DOUT], f16, kind="ExternalInput")
    out_d = nc.dram_tensor("out", [rows, DOUT], f32, kind="ExternalOutput")

    with tile.TileContext(nc) as tc:
        with tc.tile_pool(name="wpool", bufs=1) as wpool, \
             tc.tile_pool(name="xpool", bufs=4) as xpool, \
             tc.tile_pool(name="upool", bufs=3) as upool, \
             tc.tile_pool(name="mpool", bufs=3) as mpool, \
             tc.tile_pool(name="cpool", bufs=3) as cpool, \
             tc.tile_pool(name="opool", bufs=3) as opool, \
             tc.tile_pool(name="pspool", bufs=3, space="PSUM") as pspool:

            wh = wpool.tile([P, KC, DOUT], f16)
            nc.sync.dma_start(wh[:], wh_d[:])
            wl = wpool.tile([P, KC, DOUT], f16)
            nc.sync.dma_start(wl[:], wl_d[:])
            if with_bias:
                bh = wpool.tile([1, DOUT], f16)
                nc.sync.dma_start(bh[:], bh_d[:])
                bl = wpool.tile([1, DOUT], f16)
                nc.sync.dma_start(bl[:], bl_d[:])
                ones = wpool.tile([1, P], f16)
                nc.gpsimd.memset(ones[:], 1.0)

            for mt in range(n_macros):
                xh_t = xpool.tile([P, KC, MACRO], f16, tag="xh")
                nc.sync.dma_start(xh_t[:], xh_d[mt, :, :, :])
                xl_t = xpool.tile([P, KC, MACRO], f16, tag="xl")
                nc.sync.dma_start(xl_t[:], xl_d[mt, :, :, :])

                ps = pspool.tile([P, 2 * DOUT], f32)
                for s in range(2):
                    acc = ps[:, s * DOUT:(s + 1) * DOUT]
                    mms = []
                    if with_bias:
                        mms.append((ones[:, :], bh[:, :]))
                        mms.append((ones[:, :], bl[:, :]))
                    rs = slice(s * P, (s + 1) * P)
                    for (xa, wb) in ((xl_t, wh), (xh_t, wl), (xh_t, wh)):
                        for c in range(KC):
                            mms.append((xa[:, c, rs], wb[:, c, :]))
                    last = len(mms) - 1
                    for i, (lhsT, rhs) in enumerate(mms):
                        nc.tensor.matmul(acc, lhsT, rhs,
                                         start=(i == 0), stop=(i == last))

                # u = logits for 256 rows: [p, (h, j)] standard dout order
                u = upool.tile([P, 2 * DOUT], f32)
                nc.scalar.activation(u[:], ps[:], mybir.ActivationFunctionType.Copy)

                # grouped max over U=4 (groups contiguous): one fused reduce
                ug = u[:].rearrange("p (g s) -> p g s", s=U)
                m = mpool.tile([P, 2 * G], f32, tag="m")
                nc.vector.tensor_reduce(m[:], ug, axis=mybir.AxisListType.X,
                                        op=mybir.AluOpType.max)
                mb = m[:].unsqueeze(2).broadcast_to([P, 2 * G, U])

                o = opool.tile([P, 2 * DOUT], f32)
                if mask_mode == "dve_cmp":
                    # cmp on DVE (broadcast is_ge), multiply on GpSimd
                    cmp = cpool.tile([P, 2 * G, U], f32)
                    nc.vector.tensor_tensor(cmp[:], ug, mb, mybir.AluOpType.is_ge)
                    nc.gpsimd.tensor_tensor(
                        o[:], u[:], cmp[:].rearrange("p g s -> p (g s)"),
                        mybir.AluOpType.mult)
                else:
                    # d = u - max on GpSimd, then fused (d>=0)*u on DVE
                    d = cpool.tile([P, 2 * G, U], f32)
                    nc.gpsimd.tensor_tensor(d[:], ug, mb, mybir.AluOpType.subtract)
                    nc.vector.scalar_tensor_tensor(
                        o[:], d[:].rearrange("p g s -> p (g s)"), 0.0, u[:],
                        op0=mybir.AluOpType.is_ge, op1=mybir.AluOpType.mult)

                dst = out_d[mt * MACRO:(mt + 1) * MACRO, :].rearrange(
                    "(s p) j -> p s j", p=P)
                nc.sync.dma_start(dst, o[:].rearrange("p (s j) -> p s j", s=2))

    nc.compile()
    return nc


_programs: dict = {}


def _get_program(n_macros: int, with_bias: bool, mask_mode: str = "pool_sub"):
    key = (n_macros, with_bias, mask_mode)
    if key not in _programs:
        _programs[key] = build_program(n_macros, with_bias, mask_mode)
    return _programs[key]


def _split_fp16(a: np.ndarray):
    hi = a.astype(np.float16)
    lo = (a - hi.astype(np.float32)).astype(np.float16)
    return hi, lo


def _pack_b(b: np.ndarray):
    """[DOUT] fp32 -> (hi, lo) [1, DOUT] fp16."""
    return _split_fp16(np.ascontiguousarray(b.astype(np.float32).reshape(1, DOUT)))


def _pack_x(xs: np.ndarray, n_macros: int):
    """[rows, DIN] fp32 -> (hi, lo) tiled [n_macros, P, KC, MACRO] fp16."""
    hi, lo = _split_fp16(xs)
    packs = []
    for a in (hi, lo):
        # [rows, DIN] -> transpose -> k = c*P + p ; row = mt*MACRO + r
        at = np.ascontiguousarray(a.T)                      # [DIN, rows]
        at = at.reshape(KC, P, n_macros, MACRO)             # [c, p, mt, r]
        packs.append(np.ascontiguousarray(at.transpose(2, 1, 0, 3)))
    return packs


def _pack_w(W: np.ndarray):
    """[DOUT, DIN] fp32 -> (hi, lo) tiled [P, KC, DOUT] fp16 of W.T."""
    wT = W.astype(np.float32).T                             # [DIN, DOUT]
    hi, lo = _split_fp16(np.ascontiguousarray(wT))
    packs = []
    for a in (hi, lo):
        packs.append(np.ascontiguousarray(a.reshape(KC, P, DOUT).transpose(1, 0, 2)))
    return packs


def kernel(x: np.ndarray, W: np.ndarray, b: np.ndarray) -> np.ndarray:
    x = np.asarray(x, dtype=np.float32)
    W = np.asarray(W, dtype=np.float32)
    b = np.asarray(b, dtype=np.float32)
    assert x.shape == (N, DIN) and W.shape == (DOUT, DIN) and b.shape == (DOUT,)

    with_bias = bool(np.any(b))
    n_macros = ROWS // MACRO
    nc = _get_program(n_macros, with_bias)

    return _run(nc, x, W, b, with_bias, n_macros)


def _run(nc, x, W, b, with_bias, n_macros):

    wh, wl = _pack_w(W)
    in_maps = []
    for i in range(NCORES):
        xs = x[i * ROWS:(i + 1) * ROWS]
        xh, xl = _pack_x(xs, n_macros)
        im = {"xh": xh, "xl": xl, "wh": wh, "wl": wl}
        if with_bias:
            bhi, blo = _pack_b(b)
            im["bh"] = bhi
            im["bl"] = blo
        in_maps.append(im)

    res = run_bass_kernel_spmd(nc, in_maps, list(range(NCORES)))
    return np.concatenate([res.results[i]["out"] for i in range(NCORES)], axis=0)
